# revision 24
# baseline (speedup 1.0000x reference)
"""Trainium2 Bass kernel for nn_FMNet pixel-shuffle + sigmoid.

reference:  x = FV[:, 64:, :, :]                                 # [B, 64, 64, 64]
            out[b, 8i+r, 8j+c] = sigmoid(x[b, 8r+c, i, j])       # [B, 1, 512, 512]

Per core (4 batches, pure data-parallel over batch):
  - loads: 32 DMAs of 128 KiB, one per (batch, channel-octant), partition =
    (b, i2) spatial row-pair so HBM chunks are 512 B contiguous.  Split over
    two concurrent descriptor-generator streams (dynamic DMAs serialize per
    ring at ~(partitions/8)x27 GB/s): SP HWDGE carries b0/b1, the GpSimd
    SWDGE generator carries b2/b3.  One semaphore per octant (a shared
    counting sem across a long DMA stream is racy - per-engine SDMA lanes
    complete out of order).
  - compute: 8 fused ScalarE ACTIVATE(Sigmoid) ops [128 x 1024] whose
    strided input AP performs the (c', j) -> (j*8 + c') pixel-shuffle
    interleave in the same pass (~2 ns/elem; DVE/GpSimd strided copies are
    ~4.4 ns/elem).  A dummy 1-element sigmoid up front pulls the 1.3 us
    ACT_TABLE_LOAD off the critical path.
  - stores: r-quarter waves of 256 KiB on the SP ring as soon as the two
    ACTs they need are done; the last two r-groups go out as 128 KiB
    single-r stores, with r=7's b2/b3 pair issued from the ACT HWDGE ring
    (free once the ACTIVATEs finish) to shorten the tail.
Measured ~40-43 us HW exec per core (vs ~23 us HBM roofline for the 8 MiB
of traffic; ~16 us is fixed NEFF start/stop overhead in this harness).
"""

import os
import sys

if "/opt/trn_rl_repo" not in sys.path:
    sys.path.insert(0, "/opt/trn_rl_repo")

import numpy as np

import concourse.bass as bass
from concourse import mybir
from concourse.bass_utils import run_bass_kernel_spmd

N_CORES = 8
B = 32
B_LOC = B // N_CORES   # 4
H = W = 512
S = 64
NG = 8                 # channel groups (r)

LAST_EXEC_NS = None

_cached_nc = None


def _install_trace_hook():
    """Best-effort NTFF hook so BASS_TRACE=1 yields exec_time_ns."""
    try:
        import types

        import antenv

        try:
            from antenv.axon_hooks import get_axon_ntff_profile_hook  # noqa: F401

            return
        except ImportError:
            pass
        mod = types.ModuleType("antenv.axon_hooks")
        _state = {"hook": None}
        mod.set_axon_ntff_profile_hook = lambda h: _state.__setitem__("hook", h)
        mod.get_axon_ntff_profile_hook = lambda: _state["hook"]
        sys.modules["antenv.axon_hooks"] = mod
        antenv.axon_hooks = mod
        from trn_agent_boot.trn_boot import _ntff_profile_via_ctypes

        mod.set_axon_ntff_profile_hook(
            _ntff_profile_via_ctypes("/opt/axon/libaxon_pjrt.so")
        )
    except Exception:
        pass


def _build_nc():
    import contextlib

    F32 = mybir.dt.float32
    nc = bass.Bass("TRN2", num_devices=N_CORES)
    FV = nc.declare_dram_parameter("FV", [B_LOC, 128, S, S], F32, isOutput=False)
    OUT = nc.declare_dram_parameter("OUT", [B_LOC, W, H], F32, isOutput=True)

    # partition p = (b:4, i2:32); TIN_h free = (c32:32, ip, j) for channel
    # half h; TOUT_h free = (ip:2, r4:4, q:512) for r half h
    tin = [nc.alloc_sbuf_tensor(f"tin{h}", [128, 4096], F32) for h in range(2)]
    tout = [nc.alloc_sbuf_tensor(f"tout{h}", [128, 4096], F32) for h in range(2)]

    fv = FV[:]
    out = OUT[:]

    scratch = nc.alloc_sbuf_tensor("scratch", [1, 8], F32)

    def load_aps(b, g):
        """(dst, src) APs loading channel octant g of batch b (512 B chunks)."""
        h, g4 = divmod(g, 4)
        src = fv[b, 64 + 8 * g : 64 + 8 * g + 8]  # [8, 64, 64]
        src = src.rearrange("c (i2 ip) j -> i2 c (ip j)", ip=2)
        dst = tin[h].ap()[32 * b : 32 * b + 32, 1024 * g4 : 1024 * (g4 + 1)]
        return dst, src

    def store_aps(b, rq):
        """(dst, src) APs for the store of batch b, r-quarter rq."""
        h, k = divmod(rq, 2)  # tout half h, quarter k within half
        # dest rows 16*i2 + 8*ip + (2rq + r2), cols q
        dst = out[b].rearrange(
            "(i2 ip rr r2) q -> i2 ip rr (r2 q)", i2=32, ip=2, rr=4
        )[:, :, rq, :]  # [32, 2, 1024]
        src = tout[h].ap().rearrange(
            "p (ip r2 v) -> p ip r2 v", ip=2, r2=2
        )[32 * b : 32 * b + 32, :, k, :]  # [32, 2, 1024]
        return dst, src

    def store_r_aps(b, r):
        """(dst, src) APs for the single-r store of batch b (128 KiB)."""
        h, r4 = divmod(r, 4)
        # dest rows 16*i2 + 8*ip + r, cols q
        dst = out[b].rearrange("(i2 ip rr) q -> i2 ip rr q", i2=32, ip=2)[
            :, :, r, :
        ]  # [32, 2, 512]
        src = tout[h].ap().rearrange("p (ip r4 q) -> p ip r4 q", ip=2, r4=4)[
            32 * b : 32 * b + 32, :, r4, :
        ]  # [32, 2, 512]
        return dst, src

    # Two concurrent load streams: SP HWDGE carries the whole first octant
    # (earliest ACT_0 start) plus b0/b1 of octants 1-7; the GpSimd SWDGE
    # generator carries b2/b3 of octants 1-7 in parallel.
    sp_loads = [(b, g) for g in range(5) for b in (0, 1)]
    sp_loads += [(b, g) for g in range(5, NG) for b in range(B_LOC)]
    gp_loads = [(b, g) for g in range(5) for b in (2, 3)]

    with contextlib.ExitStack() as stack:
        block = stack.enter_context(nc.Block())
        sem_oct = [stack.enter_context(nc.semaphore(f"sem_o{g}")) for g in range(NG)]
        sem_act = stack.enter_context(nc.semaphore("sem_act"))
        sem_out = stack.enter_context(nc.semaphore("sem_out"))

        # Two concurrent load streams: ring-queued DMAs serialize per ring and
        # run at (partitions/8) x 27 GB/s; b0/b1 live on even SDMA engines,
        # b2/b3 on odd, so the SP HWDGE and GpSimd SWDGE streams overlap.
        @block.sync
        def _(sync: bass.BassEngine):
            for b, g in sp_loads:
                dst, src = load_aps(b, g)
                sync.dma_start(out=dst, in_=src).then_inc(sem_oct[g], 16)
            for rq in range(3):
                sync.wait_ge(sem_act, 2 * (rq + 1))
                for b in (0, 1):
                    dst, src = store_aps(b, rq)
                    sync.dma_start(out=dst, in_=src).then_inc(sem_out, 16)
            sync.wait_ge(sem_act, 7)
            for b in (0, 1):
                dst, src = store_r_aps(b, 6)
                sync.dma_start(out=dst, in_=src).then_inc(sem_out, 16)
            sync.wait_ge(sem_act, 8)
            for b in (0, 1):
                dst, src = store_r_aps(b, 7)
                sync.dma_start(out=dst, in_=src).then_inc(sem_out, 16)
            sync.wait_ge(sem_out, 16 * 20)  # 10 SP + 8 Q7 + 2 ACT stores

        @block.gpsimd
        def _(g_eng: bass.BassEngine):
            for b, g in gp_loads:
                dst, src = load_aps(b, g)
                g_eng.dma_start(out=dst, in_=src).then_inc(sem_oct[g], 16)
            # b2/b3 stores ride the SWDGE stream; gated late (sem_act >= 4+)
            # so they don't steal SDMA bandwidth from the ACT-gating loads
            for rq in range(3):
                g_eng.wait_ge(sem_act, 4 + rq)
                for b in (2, 3):
                    dst, src = store_aps(b, rq)
                    g_eng.dma_start(out=dst, in_=src).then_inc(sem_out, 16)
            g_eng.wait_ge(sem_act, 7)
            for b in (2, 3):
                dst, src = store_r_aps(b, 6)
                g_eng.dma_start(out=dst, in_=src).then_inc(sem_out, 16)

        @block.scalar
        def _(scalar: bass.BassEngine):
            # dummy op to pull ACT_TABLE_LOAD (sigmoid) off the critical path
            scalar.activation(
                scratch.ap(), scratch.ap(), mybir.ActivationFunctionType.Sigmoid
            )
            for r in range(NG):
                h, r4 = divmod(r, 4)
                scalar.wait_ge(sem_oct[r], 64)
                # in: (ip, j, c') strided read of the (c', ip, j) tile slice
                tin_v = (
                    tin[h]
                    .ap()[:, 1024 * r4 : 1024 * (r4 + 1)]
                    .rearrange("p (c ip j) -> p ip j c", c=8, ip=2)
                )
                # out: (ip, [r4], q) with q = j*8+c' contiguous
                tout_v = tout[h].ap().rearrange(
                    "p (ip r4 q) -> p ip r4 q", ip=2, r4=4
                )[:, :, r4, :]
                scalar.activation(
                    tout_v, tin_v, mybir.ActivationFunctionType.Sigmoid
                ).then_inc(sem_act, 1)
            # tail stores for b2/b3 of r=7 on the ACT HWDGE ring
            scalar.wait_ge(sem_act, NG)
            for b in (2, 3):
                dst, src = store_r_aps(b, 7)
                scalar.dma_start(out=dst, in_=src).then_inc(sem_out, 16)

    return nc


def kernel(FV, batch_size=None, W=None, H=None, **_ignored):
    global _cached_nc, LAST_EXEC_NS
    FV = np.asarray(FV, dtype=np.float32)
    assert FV.shape == (B, 128, S, S), FV.shape

    trace = bool(os.environ.get("BASS_TRACE"))
    if trace:
        _install_trace_hook()

    if _cached_nc is None:
        _cached_nc = _build_nc()
    nc = _cached_nc

    in_maps = [{"FV": FV[k * B_LOC : (k + 1) * B_LOC]} for k in range(N_CORES)]
    res = None
    for attempt in range(3):
        try:
            res = run_bass_kernel_spmd(nc, in_maps, list(range(N_CORES)), trace=trace)
            break
        except Exception:
            # occasional transient NRT_EXEC_UNIT_UNRECOVERABLE on a cold
            # device; retry after a short pause
            if attempt == 2:
                raise
            import time

            time.sleep(2.0)
    if trace:
        LAST_EXEC_NS = res.exec_time_ns

    outs = [res.results[k]["OUT"] for k in range(N_CORES)]
    full = np.concatenate(outs, axis=0)  # [32, 512, 512]
    return full[:, None, :, :].astype(np.float32)


# revision 25
# speedup vs baseline: 1.0216x; 1.0216x over previous
"""Trainium2 Bass kernel for nn_FMNet pixel-shuffle + sigmoid.

reference:  x = FV[:, 64:, :, :]                                 # [B, 64, 64, 64]
            out[b, 8i+r, 8j+c] = sigmoid(x[b, 8r+c, i, j])       # [B, 1, 512, 512]

Per core (4 batches, pure data-parallel over batch):
  - loads: 32 DMAs of 128 KiB, one per (batch, channel-octant), partition =
    (b, i2) spatial row-pair so HBM chunks are 512 B contiguous.  Split over
    two concurrent descriptor-generator streams (dynamic DMAs serialize per
    ring at ~(partitions/8)x27 GB/s): SP HWDGE carries b0/b1, the GpSimd
    SWDGE generator carries b2/b3.  One semaphore per octant (a shared
    counting sem across a long DMA stream is racy - per-engine SDMA lanes
    complete out of order).
  - compute: 8 fused ScalarE ACTIVATE(Sigmoid) ops [128 x 1024] whose
    strided input AP performs the (c', j) -> (j*8 + c') pixel-shuffle
    interleave in the same pass (~2 ns/elem; DVE/GpSimd strided copies are
    ~4.4 ns/elem).  A dummy 1-element sigmoid up front pulls the 1.3 us
    ACT_TABLE_LOAD off the critical path.
  - stores: r-quarter waves of 256 KiB on the SP ring as soon as the two
    ACTs they need are done; the last two r-groups go out as 128 KiB
    single-r stores, with r=7's b2/b3 pair issued from the ACT HWDGE ring
    (free once the ACTIVATEs finish) to shorten the tail.
Measured ~40-43 us HW exec per core (vs ~23 us HBM roofline for the 8 MiB
of traffic; ~16 us is fixed NEFF start/stop overhead in this harness).
"""

import os
import sys

if "/opt/trn_rl_repo" not in sys.path:
    sys.path.insert(0, "/opt/trn_rl_repo")

import numpy as np

import concourse.bass as bass
from concourse import mybir
from concourse.bass_utils import run_bass_kernel_spmd

N_CORES = 8
B = 32
B_LOC = B // N_CORES   # 4
H = W = 512
S = 64
NG = 8                 # channel groups (r)

LAST_EXEC_NS = None

_cached_nc = None


def _install_trace_hook():
    """Best-effort NTFF hook so BASS_TRACE=1 yields exec_time_ns."""
    try:
        import types

        import antenv

        try:
            from antenv.axon_hooks import get_axon_ntff_profile_hook  # noqa: F401

            return
        except ImportError:
            pass
        mod = types.ModuleType("antenv.axon_hooks")
        _state = {"hook": None}
        mod.set_axon_ntff_profile_hook = lambda h: _state.__setitem__("hook", h)
        mod.get_axon_ntff_profile_hook = lambda: _state["hook"]
        sys.modules["antenv.axon_hooks"] = mod
        antenv.axon_hooks = mod
        from trn_agent_boot.trn_boot import _ntff_profile_via_ctypes

        mod.set_axon_ntff_profile_hook(
            _ntff_profile_via_ctypes("/opt/axon/libaxon_pjrt.so")
        )
    except Exception:
        pass


def _build_nc():
    import contextlib

    F32 = mybir.dt.float32
    nc = bass.Bass("TRN2", num_devices=N_CORES)
    FV = nc.declare_dram_parameter("FV", [B_LOC, 128, S, S], F32, isOutput=False)
    OUT = nc.declare_dram_parameter("OUT", [B_LOC, W, H], F32, isOutput=True)

    # partition p = (b:4, i2:32); TIN_h free = (c32:32, ip, j) for channel
    # half h; TOUT_h free = (ip:2, r4:4, q:512) for r half h
    tin = [nc.alloc_sbuf_tensor(f"tin{h}", [128, 4096], F32) for h in range(2)]
    tout = [nc.alloc_sbuf_tensor(f"tout{h}", [128, 4096], F32) for h in range(2)]

    fv = FV[:]
    out = OUT[:]

    scratch = nc.alloc_sbuf_tensor("scratch", [1, 8], F32)

    def load_aps(b, g):
        """(dst, src) APs loading channel octant g of batch b (512 B chunks)."""
        h, g4 = divmod(g, 4)
        src = fv[b, 64 + 8 * g : 64 + 8 * g + 8]  # [8, 64, 64]
        src = src.rearrange("c (i2 ip) j -> i2 c (ip j)", ip=2)
        dst = tin[h].ap()[32 * b : 32 * b + 32, 1024 * g4 : 1024 * (g4 + 1)]
        return dst, src

    def store_aps(b, rq):
        """(dst, src) APs for the store of batch b, r-quarter rq."""
        h, k = divmod(rq, 2)  # tout half h, quarter k within half
        # dest rows 16*i2 + 8*ip + (2rq + r2), cols q
        dst = out[b].rearrange(
            "(i2 ip rr r2) q -> i2 ip rr (r2 q)", i2=32, ip=2, rr=4
        )[:, :, rq, :]  # [32, 2, 1024]
        src = tout[h].ap().rearrange(
            "p (ip r2 v) -> p ip r2 v", ip=2, r2=2
        )[32 * b : 32 * b + 32, :, k, :]  # [32, 2, 1024]
        return dst, src

    def store_r_aps(b, r):
        """(dst, src) APs for the single-r store of batch b (128 KiB)."""
        h, r4 = divmod(r, 4)
        # dest rows 16*i2 + 8*ip + r, cols q
        dst = out[b].rearrange("(i2 ip rr) q -> i2 ip rr q", i2=32, ip=2)[
            :, :, r, :
        ]  # [32, 2, 512]
        src = tout[h].ap().rearrange("p (ip r4 q) -> p ip r4 q", ip=2, r4=4)[
            32 * b : 32 * b + 32, :, r4, :
        ]  # [32, 2, 512]
        return dst, src

    # Two concurrent load streams: SP HWDGE carries the whole first octant
    # (earliest ACT_0 start) plus b0/b1 of octants 1-7; the GpSimd SWDGE
    # generator carries b2/b3 of octants 1-7 in parallel.
    sp_loads = [(b, g) for g in range(5) for b in (0, 1)]
    sp_loads += [(b, g) for g in range(5, NG) for b in range(B_LOC)]
    gp_loads = [(b, g) for g in range(5) for b in (2, 3)]

    with contextlib.ExitStack() as stack:
        block = stack.enter_context(nc.Block())
        sem_oct = [stack.enter_context(nc.semaphore(f"sem_o{g}")) for g in range(NG)]
        sem_act = stack.enter_context(nc.semaphore("sem_act"))
        sem_out = stack.enter_context(nc.semaphore("sem_out"))

        # Two concurrent load streams: ring-queued DMAs serialize per ring and
        # run at (partitions/8) x 27 GB/s; b0/b1 live on even SDMA engines,
        # b2/b3 on odd, so the SP HWDGE and GpSimd SWDGE streams overlap.
        @block.sync
        def _(sync: bass.BassEngine):
            for b, g in sp_loads:
                dst, src = load_aps(b, g)
                sync.dma_start(out=dst, in_=src).then_inc(sem_oct[g], 16)
            for rq in range(3):
                sync.wait_ge(sem_act, 2 * (rq + 1))
                for b in (0, 1):
                    dst, src = store_aps(b, rq)
                    sync.dma_start(out=dst, in_=src).then_inc(sem_out, 16)
            sync.wait_ge(sem_act, 7)
            for b in (0, 1):
                dst, src = store_r_aps(b, 6)
                sync.dma_start(out=dst, in_=src).then_inc(sem_out, 16)
            sync.wait_ge(sem_act, 8)
            for b in (0, 1):
                dst, src = store_r_aps(b, 7)
                sync.dma_start(out=dst, in_=src).then_inc(sem_out, 16)
            sync.wait_ge(sem_out, 16 * 20)  # 10 SP + 8 Q7 + 2 ACT stores

        @block.gpsimd
        def _(g_eng: bass.BassEngine):
            for b, g in gp_loads:
                dst, src = load_aps(b, g)
                g_eng.dma_start(out=dst, in_=src).then_inc(sem_oct[g], 16)
            # b2/b3 stores ride the SWDGE stream once its loads are queued
            for rq in range(3):
                g_eng.wait_ge(sem_act, 2 * (rq + 1))
                for b in (2, 3):
                    dst, src = store_aps(b, rq)
                    g_eng.dma_start(out=dst, in_=src).then_inc(sem_out, 16)
            g_eng.wait_ge(sem_act, 7)
            for b in (2, 3):
                dst, src = store_r_aps(b, 6)
                g_eng.dma_start(out=dst, in_=src).then_inc(sem_out, 16)

        @block.scalar
        def _(scalar: bass.BassEngine):
            # dummy op to pull ACT_TABLE_LOAD (sigmoid) off the critical path
            scalar.activation(
                scratch.ap(), scratch.ap(), mybir.ActivationFunctionType.Sigmoid
            )
            for r in range(NG):
                h, r4 = divmod(r, 4)
                scalar.wait_ge(sem_oct[r], 64)
                # in: (ip, j, c') strided read of the (c', ip, j) tile slice
                tin_v = (
                    tin[h]
                    .ap()[:, 1024 * r4 : 1024 * (r4 + 1)]
                    .rearrange("p (c ip j) -> p ip j c", c=8, ip=2)
                )
                # out: (ip, [r4], q) with q = j*8+c' contiguous
                tout_v = tout[h].ap().rearrange(
                    "p (ip r4 q) -> p ip r4 q", ip=2, r4=4
                )[:, :, r4, :]
                scalar.activation(
                    tout_v, tin_v, mybir.ActivationFunctionType.Sigmoid
                ).then_inc(sem_act, 1)
            # tail stores for b2/b3 of r=7 on the ACT HWDGE ring
            scalar.wait_ge(sem_act, NG)
            for b in (2, 3):
                dst, src = store_r_aps(b, 7)
                scalar.dma_start(out=dst, in_=src).then_inc(sem_out, 16)

    return nc


def kernel(FV, batch_size=None, W=None, H=None, **_ignored):
    global _cached_nc, LAST_EXEC_NS
    FV = np.asarray(FV, dtype=np.float32)
    assert FV.shape == (B, 128, S, S), FV.shape

    trace = bool(os.environ.get("BASS_TRACE"))
    if trace:
        _install_trace_hook()

    if _cached_nc is None:
        _cached_nc = _build_nc()
    nc = _cached_nc

    in_maps = [{"FV": FV[k * B_LOC : (k + 1) * B_LOC]} for k in range(N_CORES)]
    res = None
    for attempt in range(3):
        try:
            res = run_bass_kernel_spmd(nc, in_maps, list(range(N_CORES)), trace=trace)
            break
        except Exception:
            # occasional transient NRT_EXEC_UNIT_UNRECOVERABLE on a cold
            # device; retry after a short pause
            if attempt == 2:
                raise
            import time

            time.sleep(2.0)
    if trace:
        LAST_EXEC_NS = res.exec_time_ns

    outs = [res.results[k]["OUT"] for k in range(N_CORES)]
    full = np.concatenate(outs, axis=0)  # [32, 512, 512]
    return full[:, None, :, :].astype(np.float32)


# revision 26
# speedup vs baseline: 1.0263x; 1.0046x over previous
"""Trainium2 Bass kernel for nn_FMNet pixel-shuffle + sigmoid.

reference:  x = FV[:, 64:, :, :]                                 # [B, 64, 64, 64]
            out[b, 8i+r, 8j+c] = sigmoid(x[b, 8r+c, i, j])       # [B, 1, 512, 512]

Per core (4 batches, pure data-parallel over batch):
  - loads: 32 DMAs of 128 KiB, one per (batch, channel-octant), partition =
    (b, i2) spatial row-pair so HBM chunks are 512 B contiguous.  Split over
    two concurrent descriptor-generator streams (dynamic DMAs serialize per
    ring at ~(partitions/8)x27 GB/s): SP HWDGE carries b0/b1, the GpSimd
    SWDGE generator carries b2/b3.  One semaphore per octant (a shared
    counting sem across a long DMA stream is racy - per-engine SDMA lanes
    complete out of order).
  - compute: 8 fused ScalarE ACTIVATE(Sigmoid) ops [128 x 1024] whose
    strided input AP performs the (c', j) -> (j*8 + c') pixel-shuffle
    interleave in the same pass (~2 ns/elem; DVE/GpSimd strided copies are
    ~4.4 ns/elem).  A dummy 1-element sigmoid up front pulls the 1.3 us
    ACT_TABLE_LOAD off the critical path.
  - stores: r-quarter waves of 256 KiB on the SP ring as soon as the two
    ACTs they need are done; the last two r-groups go out as 128 KiB
    single-r stores, with r=7's b2/b3 pair issued from the ACT HWDGE ring
    (free once the ACTIVATEs finish) to shorten the tail.
Measured ~40-43 us HW exec per core (vs ~23 us HBM roofline for the 8 MiB
of traffic; ~16 us is fixed NEFF start/stop overhead in this harness).
"""

import os
import sys

if "/opt/trn_rl_repo" not in sys.path:
    sys.path.insert(0, "/opt/trn_rl_repo")

import numpy as np

import concourse.bass as bass
from concourse import mybir
from concourse.bass_utils import run_bass_kernel_spmd

N_CORES = 8
B = 32
B_LOC = B // N_CORES   # 4
H = W = 512
S = 64
NG = 8                 # channel groups (r)

LAST_EXEC_NS = None

_cached_nc = None


def _install_trace_hook():
    """Best-effort NTFF hook so BASS_TRACE=1 yields exec_time_ns."""
    try:
        import types

        import antenv

        try:
            from antenv.axon_hooks import get_axon_ntff_profile_hook  # noqa: F401

            return
        except ImportError:
            pass
        mod = types.ModuleType("antenv.axon_hooks")
        _state = {"hook": None}
        mod.set_axon_ntff_profile_hook = lambda h: _state.__setitem__("hook", h)
        mod.get_axon_ntff_profile_hook = lambda: _state["hook"]
        sys.modules["antenv.axon_hooks"] = mod
        antenv.axon_hooks = mod
        from trn_agent_boot.trn_boot import _ntff_profile_via_ctypes

        mod.set_axon_ntff_profile_hook(
            _ntff_profile_via_ctypes("/opt/axon/libaxon_pjrt.so")
        )
    except Exception:
        pass


def _build_nc():
    import contextlib

    F32 = mybir.dt.float32
    nc = bass.Bass("TRN2", num_devices=N_CORES)
    FV = nc.declare_dram_parameter("FV", [B_LOC, 128, S, S], F32, isOutput=False)
    OUT = nc.declare_dram_parameter("OUT", [B_LOC, W, H], F32, isOutput=True)

    # partition p = (b:4, i2:32); TIN_h free = (c32:32, ip, j) for channel
    # half h; TOUT_h free = (ip:2, r4:4, q:512) for r half h
    tin = [nc.alloc_sbuf_tensor(f"tin{h}", [128, 4096], F32) for h in range(2)]
    tout = [nc.alloc_sbuf_tensor(f"tout{h}", [128, 4096], F32) for h in range(2)]

    fv = FV[:]
    out = OUT[:]

    scratch = nc.alloc_sbuf_tensor("scratch", [1, 8], F32)

    def load_aps(b, g):
        """(dst, src) APs loading channel octant g of batch b (512 B chunks)."""
        h, g4 = divmod(g, 4)
        src = fv[b, 64 + 8 * g : 64 + 8 * g + 8]  # [8, 64, 64]
        src = src.rearrange("c (i2 ip) j -> i2 c (ip j)", ip=2)
        dst = tin[h].ap()[32 * b : 32 * b + 32, 1024 * g4 : 1024 * (g4 + 1)]
        return dst, src

    def store_aps(b, rq):
        """(dst, src) APs for the store of batch b, r-quarter rq."""
        h, k = divmod(rq, 2)  # tout half h, quarter k within half
        # dest rows 16*i2 + 8*ip + (2rq + r2), cols q
        dst = out[b].rearrange(
            "(i2 ip rr r2) q -> i2 ip rr (r2 q)", i2=32, ip=2, rr=4
        )[:, :, rq, :]  # [32, 2, 1024]
        src = tout[h].ap().rearrange(
            "p (ip r2 v) -> p ip r2 v", ip=2, r2=2
        )[32 * b : 32 * b + 32, :, k, :]  # [32, 2, 1024]
        return dst, src

    def store_r_aps(b, r):
        """(dst, src) APs for the single-r store of batch b (128 KiB)."""
        h, r4 = divmod(r, 4)
        # dest rows 16*i2 + 8*ip + r, cols q
        dst = out[b].rearrange("(i2 ip rr) q -> i2 ip rr q", i2=32, ip=2)[
            :, :, r, :
        ]  # [32, 2, 512]
        src = tout[h].ap().rearrange("p (ip r4 q) -> p ip r4 q", ip=2, r4=4)[
            32 * b : 32 * b + 32, :, r4, :
        ]  # [32, 2, 512]
        return dst, src

    # Two concurrent load streams: SP HWDGE carries the whole first octant
    # (earliest ACT_0 start) plus b0/b1 of octants 1-7; the GpSimd SWDGE
    # generator carries b2/b3 of octants 1-7 in parallel.
    sp_loads = [(b, g) for g in range(5) for b in (0, 1)]
    sp_loads += [(b, g) for g in range(5, NG) for b in range(B_LOC)]
    gp_loads = [(b, g) for g in range(1, 5) for b in (2, 3)]

    with contextlib.ExitStack() as stack:
        block = stack.enter_context(nc.Block())
        sem_oct = [stack.enter_context(nc.semaphore(f"sem_o{g}")) for g in range(NG)]
        sem_act = stack.enter_context(nc.semaphore("sem_act"))
        sem_out = stack.enter_context(nc.semaphore("sem_out"))

        # Two concurrent load streams: ring-queued DMAs serialize per ring and
        # run at (partitions/8) x 27 GB/s; b0/b1 live on even SDMA engines,
        # b2/b3 on odd, so the SP HWDGE and GpSimd SWDGE streams overlap.
        @block.sync
        def _(sync: bass.BassEngine):
            for b, g in sp_loads:
                dst, src = load_aps(b, g)
                sync.dma_start(out=dst, in_=src).then_inc(sem_oct[g], 16)
            for rq in range(3):
                sync.wait_ge(sem_act, 2 * (rq + 1))
                for b in (0, 1):
                    dst, src = store_aps(b, rq)
                    sync.dma_start(out=dst, in_=src).then_inc(sem_out, 16)
            sync.wait_ge(sem_act, 7)
            for b in (0, 1):
                dst, src = store_r_aps(b, 6)
                sync.dma_start(out=dst, in_=src).then_inc(sem_out, 16)
            sync.wait_ge(sem_act, 8)
            for b in (0, 1):
                dst, src = store_r_aps(b, 7)
                sync.dma_start(out=dst, in_=src).then_inc(sem_out, 16)
            sync.wait_ge(sem_out, 16 * 20)  # 10 SP + 8 Q7 + 2 ACT stores

        @block.gpsimd
        def _(g_eng: bass.BassEngine):
            for b, g in gp_loads:
                dst, src = load_aps(b, g)
                g_eng.dma_start(out=dst, in_=src).then_inc(sem_oct[g], 16)
            # b2/b3 stores ride the SWDGE stream once its loads are queued
            for rq in range(3):
                g_eng.wait_ge(sem_act, 2 * (rq + 1))
                for b in (2, 3):
                    dst, src = store_aps(b, rq)
                    g_eng.dma_start(out=dst, in_=src).then_inc(sem_out, 16)
            g_eng.wait_ge(sem_act, 7)
            for b in (2, 3):
                dst, src = store_r_aps(b, 6)
                g_eng.dma_start(out=dst, in_=src).then_inc(sem_out, 16)

        @block.scalar
        def _(scalar: bass.BassEngine):
            # oct0 b2/b3 via the idle ACT ring: earliest possible ACT_0 start
            # (the SWDGE stream's first-byte latency would gate it otherwise)
            for b in (2, 3):
                dst, src = load_aps(b, 0)
                scalar.dma_start(out=dst, in_=src).then_inc(sem_oct[0], 16)
            # dummy op to pull ACT_TABLE_LOAD (sigmoid) off the critical path
            scalar.activation(
                scratch.ap(), scratch.ap(), mybir.ActivationFunctionType.Sigmoid
            )
            for r in range(NG):
                h, r4 = divmod(r, 4)
                scalar.wait_ge(sem_oct[r], 64)
                # in: (ip, j, c') strided read of the (c', ip, j) tile slice
                tin_v = (
                    tin[h]
                    .ap()[:, 1024 * r4 : 1024 * (r4 + 1)]
                    .rearrange("p (c ip j) -> p ip j c", c=8, ip=2)
                )
                # out: (ip, [r4], q) with q = j*8+c' contiguous
                tout_v = tout[h].ap().rearrange(
                    "p (ip r4 q) -> p ip r4 q", ip=2, r4=4
                )[:, :, r4, :]
                scalar.activation(
                    tout_v, tin_v, mybir.ActivationFunctionType.Sigmoid
                ).then_inc(sem_act, 1)
            # tail stores for b2/b3 of r=7 on the ACT HWDGE ring
            scalar.wait_ge(sem_act, NG)
            for b in (2, 3):
                dst, src = store_r_aps(b, 7)
                scalar.dma_start(out=dst, in_=src).then_inc(sem_out, 16)

    return nc


def kernel(FV, batch_size=None, W=None, H=None, **_ignored):
    global _cached_nc, LAST_EXEC_NS
    FV = np.asarray(FV, dtype=np.float32)
    assert FV.shape == (B, 128, S, S), FV.shape

    trace = bool(os.environ.get("BASS_TRACE"))
    if trace:
        _install_trace_hook()

    if _cached_nc is None:
        _cached_nc = _build_nc()
    nc = _cached_nc

    in_maps = [{"FV": FV[k * B_LOC : (k + 1) * B_LOC]} for k in range(N_CORES)]
    res = None
    for attempt in range(3):
        try:
            res = run_bass_kernel_spmd(nc, in_maps, list(range(N_CORES)), trace=trace)
            break
        except Exception:
            # occasional transient NRT_EXEC_UNIT_UNRECOVERABLE on a cold
            # device; retry after a short pause
            if attempt == 2:
                raise
            import time

            time.sleep(2.0)
    if trace:
        LAST_EXEC_NS = res.exec_time_ns

    outs = [res.results[k]["OUT"] for k in range(N_CORES)]
    full = np.concatenate(outs, axis=0)  # [32, 512, 512]
    return full[:, None, :, :].astype(np.float32)


# revision 27
# speedup vs baseline: 1.0406x; 1.0140x over previous
"""Trainium2 Bass kernel for nn_FMNet pixel-shuffle + sigmoid.

reference:  x = FV[:, 64:, :, :]                                 # [B, 64, 64, 64]
            out[b, 8i+r, 8j+c] = sigmoid(x[b, 8r+c, i, j])       # [B, 1, 512, 512]

Per core (4 batches, pure data-parallel over batch):
  - loads: 32 DMAs of 128 KiB, one per (batch, channel-octant), partition =
    (b, i2) spatial row-pair so HBM chunks are 512 B contiguous.  Split over
    two concurrent descriptor-generator streams (dynamic DMAs serialize per
    ring at ~(partitions/8)x27 GB/s): SP HWDGE carries b0/b1, the GpSimd
    SWDGE generator carries b2/b3.  One semaphore per octant (a shared
    counting sem across a long DMA stream is racy - per-engine SDMA lanes
    complete out of order).
  - compute: 8 fused ScalarE ACTIVATE(Sigmoid) ops [128 x 1024] whose
    strided input AP performs the (c', j) -> (j*8 + c') pixel-shuffle
    interleave in the same pass (~2 ns/elem; DVE/GpSimd strided copies are
    ~4.4 ns/elem).  A dummy 1-element sigmoid up front pulls the 1.3 us
    ACT_TABLE_LOAD off the critical path.
  - stores: r-quarter waves of 256 KiB on the SP ring as soon as the two
    ACTs they need are done; the last two r-groups go out as 128 KiB
    single-r stores, with r=7's b2/b3 pair issued from the ACT HWDGE ring
    (free once the ACTIVATEs finish) to shorten the tail.
Measured ~40-43 us HW exec per core (vs ~23 us HBM roofline for the 8 MiB
of traffic; ~16 us is fixed NEFF start/stop overhead in this harness).
"""

import os
import sys

if "/opt/trn_rl_repo" not in sys.path:
    sys.path.insert(0, "/opt/trn_rl_repo")

import numpy as np

import concourse.bass as bass
from concourse import mybir
from concourse.bass_utils import run_bass_kernel_spmd

N_CORES = 8
B = 32
B_LOC = B // N_CORES   # 4
H = W = 512
S = 64
NG = 8                 # channel groups (r)

LAST_EXEC_NS = None

_cached_nc = None


def _install_trace_hook():
    """Best-effort NTFF hook so BASS_TRACE=1 yields exec_time_ns."""
    try:
        import types

        import antenv

        try:
            from antenv.axon_hooks import get_axon_ntff_profile_hook  # noqa: F401

            return
        except ImportError:
            pass
        mod = types.ModuleType("antenv.axon_hooks")
        _state = {"hook": None}
        mod.set_axon_ntff_profile_hook = lambda h: _state.__setitem__("hook", h)
        mod.get_axon_ntff_profile_hook = lambda: _state["hook"]
        sys.modules["antenv.axon_hooks"] = mod
        antenv.axon_hooks = mod
        from trn_agent_boot.trn_boot import _ntff_profile_via_ctypes

        mod.set_axon_ntff_profile_hook(
            _ntff_profile_via_ctypes("/opt/axon/libaxon_pjrt.so")
        )
    except Exception:
        pass


def _build_nc():
    import contextlib

    F32 = mybir.dt.float32
    nc = bass.Bass("TRN2", num_devices=N_CORES)
    FV = nc.declare_dram_parameter("FV", [B_LOC, 128, S, S], F32, isOutput=False)
    OUT = nc.declare_dram_parameter("OUT", [B_LOC, W, H], F32, isOutput=True)

    # partition p = (b:4, i2:32); TIN_h free = (c32:32, ip, j) for channel
    # half h; TOUT_h free = (ip:2, r4:4, q:512) for r half h
    tin = [nc.alloc_sbuf_tensor(f"tin{h}", [128, 4096], F32) for h in range(2)]
    tout = [nc.alloc_sbuf_tensor(f"tout{h}", [128, 4096], F32) for h in range(2)]

    fv = FV[:]
    out = OUT[:]

    scratch = nc.alloc_sbuf_tensor("scratch", [1, 8], F32)

    def load_aps(b, g):
        """(dst, src) APs loading channel octant g of batch b (512 B chunks)."""
        h, g4 = divmod(g, 4)
        src = fv[b, 64 + 8 * g : 64 + 8 * g + 8]  # [8, 64, 64]
        src = src.rearrange("c (i2 ip) j -> i2 c (ip j)", ip=2)
        dst = tin[h].ap()[32 * b : 32 * b + 32, 1024 * g4 : 1024 * (g4 + 1)]
        return dst, src

    def store_aps(b, rq):
        """(dst, src) APs for the store of batch b, r-quarter rq."""
        h, k = divmod(rq, 2)  # tout half h, quarter k within half
        # dest rows 16*i2 + 8*ip + (2rq + r2), cols q
        dst = out[b].rearrange(
            "(i2 ip rr r2) q -> i2 ip rr (r2 q)", i2=32, ip=2, rr=4
        )[:, :, rq, :]  # [32, 2, 1024]
        src = tout[h].ap().rearrange(
            "p (ip r2 v) -> p ip r2 v", ip=2, r2=2
        )[32 * b : 32 * b + 32, :, k, :]  # [32, 2, 1024]
        return dst, src

    def store_r_aps(b, r):
        """(dst, src) APs for the single-r store of batch b (128 KiB)."""
        h, r4 = divmod(r, 4)
        # dest rows 16*i2 + 8*ip + r, cols q
        dst = out[b].rearrange("(i2 ip rr) q -> i2 ip rr q", i2=32, ip=2)[
            :, :, r, :
        ]  # [32, 2, 512]
        src = tout[h].ap().rearrange("p (ip r4 q) -> p ip r4 q", ip=2, r4=4)[
            32 * b : 32 * b + 32, :, r4, :
        ]  # [32, 2, 512]
        return dst, src

    # Two concurrent load streams: SP HWDGE carries the whole first octant
    # (earliest ACT_0 start) plus b0/b1 of octants 1-7; the GpSimd SWDGE
    # generator carries b2/b3 of octants 1-7 in parallel.
    sp_loads = [(b, g) for g in range(6) for b in (0, 1)]
    sp_loads += [(b, g) for g in range(6, NG) for b in range(B_LOC)]
    gp_loads = [(b, g) for g in range(1, 6) for b in (2, 3)]

    with contextlib.ExitStack() as stack:
        block = stack.enter_context(nc.Block())
        sem_oct = [stack.enter_context(nc.semaphore(f"sem_o{g}")) for g in range(NG)]
        sem_act = stack.enter_context(nc.semaphore("sem_act"))
        sem_out = stack.enter_context(nc.semaphore("sem_out"))

        # Two concurrent load streams: ring-queued DMAs serialize per ring and
        # run at (partitions/8) x 27 GB/s; b0/b1 live on even SDMA engines,
        # b2/b3 on odd, so the SP HWDGE and GpSimd SWDGE streams overlap.
        @block.sync
        def _(sync: bass.BassEngine):
            for b, g in sp_loads:
                dst, src = load_aps(b, g)
                sync.dma_start(out=dst, in_=src).then_inc(sem_oct[g], 16)
            for rq in range(3):
                sync.wait_ge(sem_act, 2 * (rq + 1))
                for b in (0, 1):
                    dst, src = store_aps(b, rq)
                    sync.dma_start(out=dst, in_=src).then_inc(sem_out, 16)
            sync.wait_ge(sem_act, 7)
            for b in (0, 1):
                dst, src = store_r_aps(b, 6)
                sync.dma_start(out=dst, in_=src).then_inc(sem_out, 16)
            sync.wait_ge(sem_act, 8)
            for b in (0, 1):
                dst, src = store_r_aps(b, 7)
                sync.dma_start(out=dst, in_=src).then_inc(sem_out, 16)
            sync.wait_ge(sem_out, 16 * 20)  # 10 SP + 8 Q7 + 2 ACT stores

        @block.gpsimd
        def _(g_eng: bass.BassEngine):
            for b, g in gp_loads:
                dst, src = load_aps(b, g)
                g_eng.dma_start(out=dst, in_=src).then_inc(sem_oct[g], 16)
            # b2/b3 stores ride the SWDGE stream once its loads are queued
            for rq in range(3):
                g_eng.wait_ge(sem_act, 2 * (rq + 1))
                for b in (2, 3):
                    dst, src = store_aps(b, rq)
                    g_eng.dma_start(out=dst, in_=src).then_inc(sem_out, 16)
            g_eng.wait_ge(sem_act, 7)
            for b in (2, 3):
                dst, src = store_r_aps(b, 6)
                g_eng.dma_start(out=dst, in_=src).then_inc(sem_out, 16)

        @block.scalar
        def _(scalar: bass.BassEngine):
            # oct0 b2/b3 via the idle ACT ring: earliest possible ACT_0 start
            # (the SWDGE stream's first-byte latency would gate it otherwise)
            for b in (2, 3):
                dst, src = load_aps(b, 0)
                scalar.dma_start(out=dst, in_=src).then_inc(sem_oct[0], 16)
            # dummy op to pull ACT_TABLE_LOAD (sigmoid) off the critical path
            scalar.activation(
                scratch.ap(), scratch.ap(), mybir.ActivationFunctionType.Sigmoid
            )
            for r in range(NG):
                h, r4 = divmod(r, 4)
                scalar.wait_ge(sem_oct[r], 64)
                # in: (ip, j, c') strided read of the (c', ip, j) tile slice
                tin_v = (
                    tin[h]
                    .ap()[:, 1024 * r4 : 1024 * (r4 + 1)]
                    .rearrange("p (c ip j) -> p ip j c", c=8, ip=2)
                )
                # out: (ip, [r4], q) with q = j*8+c' contiguous
                tout_v = tout[h].ap().rearrange(
                    "p (ip r4 q) -> p ip r4 q", ip=2, r4=4
                )[:, :, r4, :]
                scalar.activation(
                    tout_v, tin_v, mybir.ActivationFunctionType.Sigmoid
                ).then_inc(sem_act, 1)
            # tail stores for b2/b3 of r=7 on the ACT HWDGE ring
            scalar.wait_ge(sem_act, NG)
            for b in (2, 3):
                dst, src = store_r_aps(b, 7)
                scalar.dma_start(out=dst, in_=src).then_inc(sem_out, 16)

    return nc


def kernel(FV, batch_size=None, W=None, H=None, **_ignored):
    global _cached_nc, LAST_EXEC_NS
    FV = np.asarray(FV, dtype=np.float32)
    assert FV.shape == (B, 128, S, S), FV.shape

    trace = bool(os.environ.get("BASS_TRACE"))
    if trace:
        _install_trace_hook()

    if _cached_nc is None:
        _cached_nc = _build_nc()
    nc = _cached_nc

    in_maps = [{"FV": FV[k * B_LOC : (k + 1) * B_LOC]} for k in range(N_CORES)]
    res = None
    for attempt in range(3):
        try:
            res = run_bass_kernel_spmd(nc, in_maps, list(range(N_CORES)), trace=trace)
            break
        except Exception:
            # occasional transient NRT_EXEC_UNIT_UNRECOVERABLE on a cold
            # device; retry after a short pause
            if attempt == 2:
                raise
            import time

            time.sleep(2.0)
    if trace:
        LAST_EXEC_NS = res.exec_time_ns

    outs = [res.results[k]["OUT"] for k in range(N_CORES)]
    full = np.concatenate(outs, axis=0)  # [32, 512, 512]
    return full[:, None, :, :].astype(np.float32)
